# revision 10
# baseline (speedup 1.0000x reference)
"""HGN layer (gnn_message_passing) Bass kernel for 8 TRN2 NeuronCores.

Sharding: E axis (2048) split 256 rows/core; adjacencies sharded on axis 1
(output rows i). R=24 replicated. Cross-core reductions (E-axis softmax
denominators, r_prime partial sums, scrambled-alpha table) use one AllGather
plus one AllReduce of tiny buffers.

Key algebraic fold: ent message = sum_r alpha[r,i] * (adj_r @ ent)[i,:]
                  = (sum_r alpha[r,i] * adj_r[i,:]) @ ent
so the 24 big matmuls collapse into an elementwise fold (ACT mult + DVE add
while adjacency tiles stream from HBM) followed by ONE [256,2048]@[2048,128]
matmul per core. lin_b is dropped: it shifts every logit of a softmax
equally, so the output is invariant to it.
"""

import numpy as np

E, R, F, NC = 2048, 24, 128, 8
S = E // NC          # 256 entity rows per core
ET = S // 128        # 2 partition tiles per core
JT = E // 128        # 16 contraction tiles

_CACHE = {}


def _build():
    from concourse import bass, bacc, tile, mybir

    fp32 = mybir.dt.float32
    AF = mybir.ActivationFunctionType
    ALU = mybir.AluOpType
    RG = [list(range(NC))]

    nc = bacc.Bacc("TRN2", target_bir_lowering=False, debug=False, num_devices=NC)

    adj_d = nc.dram_tensor("adj", [R, S, E], fp32, kind="ExternalInput")
    ent_d = nc.dram_tensor("ent", [E, F], fp32, kind="ExternalInput")
    entsl_d = nc.dram_tensor("ent_slice", [S, F], fp32, kind="ExternalInput")
    rel_d = nc.dram_tensor("rel", [R, F], fp32, kind="ExternalInput")
    relf_d = nc.dram_tensor("rel_flat", [1, R * F], fp32, kind="ExternalInput")
    A_d = nc.dram_tensor("A_slice", [S, R], fp32, kind="ExternalInput")
    went_d = nc.dram_tensor("w_ent", [F, F], fp32, kind="ExternalInput")
    wrel_d = nc.dram_tensor("w_rel", [F, F], fp32, kind="ExternalInput")
    linw_d = nc.dram_tensor("lin_w", [1, F], fp32, kind="ExternalInput")
    eout_d = nc.dram_tensor("ent_out", [S, F], fp32, kind="ExternalOutput")
    rout_d = nc.dram_tensor("rel_out", [R, F], fp32, kind="ExternalOutput")
    ident_d = nc.inline_tensor(np.eye(128, dtype=np.float32), name="ident")

    with tile.TileContext(nc) as tc:
        with (
            tc.tile_pool(name="consts", bufs=1) as cp,
            tc.tile_pool(name="adjp", bufs=12) as adjp,
            tc.tile_pool(name="big", bufs=1) as bigp,
            tc.tile_pool(name="scr", bufs=2) as scr,
            tc.tile_pool(name="flat", bufs=1) as flp,
            tc.tile_pool(name="ps_s", bufs=1, space="PSUM") as ps_s,
            tc.tile_pool(name="ps_b", bufs=1, space="PSUM") as ps_b,
            tc.tile_pool(name="ps_m", bufs=2, space="PSUM") as ps_m,
            tc.tile_pool(name="ps_msg", bufs=2, space="PSUM") as ps_msg,
            tc.tile_pool(name="dram", bufs=1, space="DRAM") as dp,
        ):
            dma = nc.sync.dma_start
            tt = nc.vector.tensor_tensor

            # ---- constants ----
            id_sb = cp.tile([128, 128], fp32, tag="id")
            dma(out=id_sb[:], in_=ident_d[:])
            ones_c = cp.tile([128, 1], fp32, tag="onc")
            nc.vector.memset(ones_c[:], 1.0)
            ones_r = cp.tile([1, 128], fp32, tag="onr")
            nc.vector.memset(ones_r[:], 1.0)
            ent_sb = cp.tile([128, E], fp32, tag="entf")
            for jt in range(JT):
                dma(out=ent_sb[:, jt * 128:(jt + 1) * 128],
                    in_=ent_d[jt * 128:(jt + 1) * 128, :])
            ent_et = []
            for et in range(ET):
                t = cp.tile([128, F], fp32, tag=f"ent{et}")
                dma(out=t[:], in_=entsl_d[et * 128:(et + 1) * 128, :])
                ent_et.append(t)
            rel_sb = cp.tile([R, F], fp32, tag="rel")
            dma(out=rel_sb[:], in_=rel_d[:])
            relf_sb = flp.tile([1, R * F], fp32, tag="flat12k")
            dma(out=relf_sb[:], in_=relf_d[:])
            A_et = []
            for et in range(ET):
                t = cp.tile([128, R], fp32, tag=f"A{et}")
                dma(out=t[:], in_=A_d[et * 128:(et + 1) * 128, :])
                A_et.append(t)
            went_sb = cp.tile([F, F], fp32, tag="went")
            dma(out=went_sb[:], in_=went_d[:])
            wrel_sb = cp.tile([F, F], fp32, tag="wrel")
            dma(out=wrel_sb[:], in_=wrel_d[:])
            linw_sb = cp.tile([1, F], fp32, tag="linw")
            dma(out=linw_sb[:], in_=linw_d[:])

            # ---- adjacency stream (issued first; consumed by the fold) ----
            adj_t = {}
            for et in range(ET):
                for r in range(R):
                    t = adjp.tile([128, E], fp32, tag="adj")
                    dma(out=t[:], in_=adj_d[r, et * 128:(et + 1) * 128, :])
                    adj_t[(et, r)] = t

            # ---- adjn = softmax(A, axis=1) ----
            adjn_et = []
            for et in range(ET):
                eA = scr.tile([128, R], fp32, tag="eA")
                nc.scalar.activation(eA[:], A_et[et][:], AF.Exp)
                s = scr.tile([128, 1], fp32, tag="sm1")
                nc.vector.reduce_sum(out=s[:], in_=eA[:], axis=mybir.AxisListType.X)
                inv = scr.tile([128, 1], fp32, tag="sm2")
                nc.vector.reciprocal(inv[:], s[:])
                an = cp.tile([128, R], fp32, tag=f"adjn{et}")
                nc.vector.tensor_scalar(an[:], eA[:], inv[:], None, op0=ALU.mult)
                adjn_et.append(an)

            # ---- rel broadcast [128, R*F] and lin_w broadcast [128, F] ----
            relb = bigp.tile([128, R * F], fp32, tag="bc3072")
            for c in range(6):
                pb = ps_b.tile([128, 512], fp32, tag="psb")
                nc.tensor.matmul(pb[:], ones_r[:], relf_sb[:, c * 512:(c + 1) * 512],
                                 start=True, stop=True)
                nc.scalar.copy(relb[:, c * 512:(c + 1) * 512], pb[:])
            linwb = cp.tile([128, F], fp32, tag="linwb")
            pm = ps_m.tile([128, 128], fp32, tag="psm")
            nc.tensor.matmul(pm[:], ones_r[:], linw_sb[:], start=True, stop=True)
            nc.vector.tensor_copy(linwb[:], pm[:])

            # ---- X = exp(leakyrelu(adjn * ent * rel)), D_R = sum_r X ----
            X_et, DR_et = [], []
            for et in range(ET):
                X = bigp.tile([128, R * F], fp32, tag=f"X{et}")
                DR = cp.tile([128, F], fp32, tag=f"DR{et}")
                for r in range(R):
                    rs = slice(r * 128, (r + 1) * 128)
                    t1 = scr.tile([128, 128], fp32, tag="t1")
                    tt(out=t1[:], in0=ent_et[et][:], in1=relb[:, rs], op=ALU.mult)
                    nc.scalar.activation(X[:, rs], t1[:], AF.Lrelu,
                                         scale=adjn_et[et][:, r:r + 1], alpha=0.2)
                    nc.scalar.activation(X[:, rs], X[:, rs], AF.Exp)
                    if r == 0:
                        nc.vector.tensor_copy(DR[:], X[:, rs])
                    else:
                        tt(out=DR[:], in0=DR[:], in1=X[:, rs], op=ALU.add)
                X_et.append(X)
                DR_et.append(DR)

            # ---- alpha_pre[e,r] = sum_f (X/D_R) * lin_w  -> AllGather ----
            invdr_et, ap_et = [], []
            for et in range(ET):
                invdr = cp.tile([128, F], fp32, tag=f"invdr{et}")
                nc.vector.reciprocal(invdr[:], DR_et[et][:])
                invdr_et.append(invdr)
                C = scr.tile([128, F], fp32, tag=f"C{et}")
                tt(out=C[:], in0=invdr[:], in1=linwb[:], op=ALU.mult)
                apre = cp.tile([128, R], fp32, tag=f"apre{et}")
                for r in range(R):
                    rs = slice(r * 128, (r + 1) * 128)
                    z = scr.tile([128, 128], fp32, tag="z")
                    tt(out=z[:], in0=X_et[et][:, rs], in1=C[:], op=ALU.mult)
                    nc.vector.reduce_sum(out=apre[:, r:r + 1], in_=z[:],
                                         axis=mybir.AxisListType.X)
                ap_et.append(apre)
            ag_in = dp.tile([S, R], fp32, tag="agin")
            ag_out = dp.tile([R, E], fp32, tag="agout")
            for et in range(ET):
                dma(out=ag_in[et * 128:(et + 1) * 128, :], in_=ap_et[et][:])
            nc.gpsimd.collective_compute("AllGather", ALU.bypass, replica_groups=RG,
                                         ins=[ag_in.opt()], outs=[ag_out.opt()])

            # ---- D_E and r_prime partials (flat, partition 0) -> AllReduce ----
            cc_in = dp.tile([1, R * F], fp32, tag="ccin")
            cc_out = dp.tile([1, R * F], fp32, tag="ccout")
            cc2_in = dp.tile([R, F], fp32, tag="cc2in")
            cc2_out = dp.tile([R, F], fp32, tag="cc2out")
            for r in range(R):
                rs = slice(r * 128, (r + 1) * 128)
                pde = ps_s.tile([1, 128], fp32, tag="pde")
                prp = ps_s.tile([1, 128], fp32, tag="prp")
                for et in range(ET):
                    nc.tensor.matmul(pde[:], ones_c[:], X_et[et][:, rs],
                                     start=(et == 0), stop=(et == ET - 1))
                for et in range(ET):
                    ar = scr.tile([128, 128], fp32, tag="ar")
                    tt(out=ar[:], in0=X_et[et][:, rs], in1=invdr_et[et][:], op=ALU.mult)
                    nc.tensor.matmul(prp[:], ones_c[:], ar[:],
                                     start=(et == 0), stop=(et == ET - 1))
                sde = scr.tile([1, 128], fp32, tag="sde")
                nc.scalar.copy(sde[:], pde[:])
                dma(out=cc_in[0:1, rs], in_=sde[:])
                srp = scr.tile([1, 128], fp32, tag="srp")
                nc.scalar.copy(srp[:], prp[:])
                dma(out=cc2_in[r:r + 1, :], in_=srp[:])
            nc.gpsimd.collective_compute("AllReduce", ALU.add, replica_groups=RG,
                                         ins=[cc_in.opt()], outs=[cc_out.opt()])
            nc.gpsimd.collective_compute("AllReduce", ALU.add, replica_groups=RG,
                                         ins=[cc2_in.opt()], outs=[cc2_out.opt()])

            # ---- scrambled alpha table: softmax rows of [R, E] view ----
            from concourse.bass import ts as dyn_ts
            tab = cp.tile([R, E], fp32, tag="tab")
            dma(out=tab[:], in_=ag_out[:])
            nc.scalar.activation(tab[:], tab[:], AF.Exp)
            den = scr.tile([R, 1], fp32, tag="den")
            nc.vector.reduce_sum(out=den[:], in_=tab[:], axis=mybir.AxisListType.X)
            invden = scr.tile([R, 1], fp32, tag="invden")
            nc.vector.reciprocal(invden[:], den[:])
            own = cp.tile([R, S], fp32, tag="own")
            pid = nc.partition_id()
            dma(out=own[:], in_=tab[:, dyn_ts(pid, S)])
            aln = cp.tile([R, S], fp32, tag="aln")
            nc.vector.tensor_scalar(aln[:], own[:], invden[:], None, op0=ALU.mult)
            alphaT_et = []
            for et in range(ET):
                ptp = ps_m.tile([128, 128], fp32, tag="psm")
                nc.tensor.transpose(ptp[:, 0:R], aln[:, et * 128:(et + 1) * 128],
                                    id_sb[0:R, 0:R])
                at = cp.tile([128, R], fp32, tag=f"alT{et}")
                nc.vector.tensor_copy(at[:], ptp[:, 0:R])
                alphaT_et.append(at)

            # ---- invD_E broadcast [128, R*F] ----
            invde = flp.tile([1, R * F], fp32, tag="flat12k")
            dma(out=invde[:], in_=cc_out[:])
            nc.vector.reciprocal(invde[:], invde[:])
            invdeb = bigp.tile([128, R * F], fp32, tag="bc3072")
            for c in range(6):
                pb2 = ps_b.tile([128, 512], fp32, tag="psb")
                nc.tensor.matmul(pb2[:], ones_r[:], invde[:, c * 512:(c + 1) * 512],
                                 start=True, stop=True)
                nc.scalar.copy(invdeb[:, c * 512:(c + 1) * 512], pb2[:])

            # ---- h_prime = (sum_r X * invD_E) * ent ----
            hp_et = []
            for et in range(ET):
                acc = scr.tile([128, F], fp32, tag=f"hacc{et}")
                for r in range(R):
                    rs = slice(r * 128, (r + 1) * 128)
                    m = scr.tile([128, 128], fp32, tag="hm")
                    tt(out=m[:], in0=X_et[et][:, rs], in1=invdeb[:, rs], op=ALU.mult)
                    if r == 0:
                        nc.vector.tensor_copy(acc[:], m[:])
                    else:
                        tt(out=acc[:], in0=acc[:], in1=m[:], op=ALU.add)
                hp = cp.tile([128, F], fp32, tag=f"hp{et}")
                tt(out=hp[:], in0=acc[:], in1=ent_et[et][:], op=ALU.mult)
                hp_et.append(hp)

            # ---- rel_output = (r_prime * rel) @ w_rel ----
            rpS = cp.tile([R, F], fp32, tag="rpS")
            dma(out=rpS[:], in_=cc2_out[:])
            rpm = cp.tile([R, F], fp32, tag="rpm")
            tt(out=rpm[:], in0=rpS[:], in1=rel_sb[:], op=ALU.mult)
            ptr = ps_m.tile([128, 128], fp32, tag="psm")
            nc.tensor.transpose(ptr[:, 0:R], rpm[:], id_sb[0:R, 0:R])
            rpT = cp.tile([128, R], fp32, tag="rpT")
            nc.vector.tensor_copy(rpT[:], ptr[:, 0:R])
            pro = ps_m.tile([128, 128], fp32, tag="psm")
            nc.tensor.matmul(pro[0:R, :], rpT[:], wrel_sb[:], start=True, stop=True)
            ro = cp.tile([R, F], fp32, tag="ro")
            nc.vector.tensor_copy(ro[:], pro[0:R, :])
            dma(out=rout_d[:], in_=ro[:])

            # ---- fold: B = sum_r alpha[r, own] * adj_r ----
            B_et = []
            for et in range(ET):
                B = bigp.tile([128, E], fp32, tag=f"B{et}")
                for r in range(R):
                    if r == 0:
                        nc.vector.tensor_scalar(B[:], adj_t[(et, 0)][:],
                                                alphaT_et[et][:, 0:1], None,
                                                op0=ALU.mult)
                    else:
                        a = adj_t[(et, r)]
                        nc.scalar.mul(a[:], a[:], alphaT_et[et][:, r:r + 1])
                        tt(out=B[:], in0=B[:], in1=a[:], op=ALU.add)
                B_et.append(B)

            # ---- msg = B @ ent ; ent_out = h_prime + msg @ w_ent ----
            for et in range(ET):
                pmsg = ps_msg.tile([128, 128], fp32, tag="pmsg")
                for jt in range(JT):
                    js = slice(jt * 128, (jt + 1) * 128)
                    ptp = ps_m.tile([128, 128], fp32, tag="psm")
                    nc.tensor.transpose(ptp[:], B_et[et][:, js], id_sb[:])
                    bt = scr.tile([128, 128], fp32, tag="bt")
                    nc.vector.tensor_copy(bt[:], ptp[:])
                    nc.tensor.matmul(pmsg[:], bt[:], ent_sb[:, js],
                                     start=(jt == 0), stop=(jt == JT - 1))
                msg = scr.tile([128, F], fp32, tag="msg")
                nc.vector.tensor_copy(msg[:], pmsg[:])
                ptm = ps_m.tile([128, 128], fp32, tag="psm")
                nc.tensor.transpose(ptm[:], msg[:], id_sb[:])
                msgT = scr.tile([128, F], fp32, tag="msgT")
                nc.vector.tensor_copy(msgT[:], ptm[:])
                pf = ps_m.tile([128, 128], fp32, tag="psm")
                nc.tensor.matmul(pf[:], msgT[:], went_sb[:], start=True, stop=True)
                osb = scr.tile([128, F], fp32, tag="osb")
                tt(out=osb[:], in0=hp_et[et][:], in1=pf[:], op=ALU.add)
                dma(out=eout_d[et * 128:(et + 1) * 128, :], in_=osb[:])

    nc.compile()
    return nc


def kernel(ent_mat, rel_mat, adjacencies, A, weight_ent, weight_rel, lin_w, lin_b):
    from concourse.bass_utils import run_bass_kernel_spmd

    if "nc" not in _CACHE:
        _CACHE["nc"] = _build()
    nc = _CACHE["nc"]

    f32 = np.float32
    ent = np.ascontiguousarray(ent_mat, dtype=f32)
    rel = np.ascontiguousarray(rel_mat, dtype=f32)
    adj = np.ascontiguousarray(adjacencies, dtype=f32)
    A_ = np.ascontiguousarray(A, dtype=f32)
    in_maps = []
    for c in range(NC):
        sl = slice(c * S, (c + 1) * S)
        in_maps.append({
            "adj": np.ascontiguousarray(adj[:, sl, :]),
            "ent": ent,
            "ent_slice": np.ascontiguousarray(ent[sl]),
            "rel": rel,
            "rel_flat": rel.reshape(1, R * F).copy(),
            "A_slice": np.ascontiguousarray(A_[sl]),
            "w_ent": np.ascontiguousarray(weight_ent, dtype=f32),
            "w_rel": np.ascontiguousarray(weight_rel, dtype=f32),
            "lin_w": np.asarray(lin_w, dtype=f32).reshape(1, F).copy(),
        })
    res = run_bass_kernel_spmd(nc, in_maps, core_ids=list(range(NC)))
    outs = res.results
    ent_out = np.concatenate([outs[c]["ent_out"] for c in range(NC)], axis=0)
    rel_out = outs[0]["rel_out"]
    return ent_out, rel_out
